# revision 1
# baseline (speedup 1.0000x reference)
"""Trainium2 Bass kernel for CrossTokenMLPAggregator (top-k masked attention aggregation).

Computes, for full inputs
    mlp_hidden   [B=2, T=2048, H=1024] f32
    attn_weights [B=2, Hh=16, T=2048, T=2048] f32
the reference:
    W = attn_weights.mean(axis=1)              # [B, T, T]
    keep top-8 per query row, renormalize kept mass to sum 1
    out = einsum('bts,bsh->bth', W_sparse, mlp_hidden)

Sharding: 8 cores, each owns 512 query rows (core c -> batch c//4,
query rows (c%4)*512 ...). Each core streams its [16, 512, 2048] slice of
attn_weights (the 512 MiB input dominates; the split is exact), sums the
heads sequentially on DVE (bit-exact with the reference's mean
accumulation order, so the top-8 selection matches exactly), finds
the top-8 with the DVE max8 instruction, masks with (W >= v8)*W in one
scalar_tensor_tensor op, transposes the masked rows on the TensorEngine
and contracts in bf16 (fp32 PSUM accumulate; ~0.2% rounding, far inside
the 2e-2 budget) against the bf16 mlp_hidden slice resident in SBUF.
Renormalization (1/kept fp32 mass) rides the PSUM->SBUF eviction on the
ScalarEngine.

Schedule notes (the kernel is DMA-bound at ~420 GB/s/core sustained):
- Per-tile epilogues (mask/transpose/matmul/store) are software-pipelined
  one tile late so the add chain — whose head-buffer releases pace the
  attn stream — never pauses at tile boundaries.
- The mlp load is staged fp32 via the Sync HWDGE queue and cast to bf16
  on the ScalarEngine in groups of 4 chunks interleaved into the first
  tiles' add chains: a group of <= stage-bufs triggers never waits on the
  casts, so head DMAs behind it in the queue are never head-of-line
  blocked (SWDGE cast-DMA and a monolithic staged load both stall the
  stream measurably).
- Output DMAs issue from the ScalarEngine's HWDGE queue right after the
  eviction they depend on, keeping the Sync queue free for head DMAs.
- The last tile streams in s-halves (its first-half top-8 scan and the
  previous tile's epilogue hide under the second half's streaming); idle
  TensorEngine cycles during that stretch get keep-warm transposes so the
  HAM clock gate doesn't re-throttle the PE to 1.2 GHz right before the
  tail matmul block; the tail matmuls run nh-major so the first output
  half stores while the second computes.
"""

import numpy as np

B, T, H, Hh, K = 2, 2048, 1024, 16, 8
NCORES = 8
QPC = (B * T) // NCORES          # 512 query rows per core
P = 128                          # partitions
TQ_TILES = QPC // P              # 4 tiles of 128 query rows
S_CHUNKS = T // P                # 16 contraction chunks
EPS_SUM = np.float32(1e-8) * np.float32(16.0)  # EPS in head-sum domain

_compiled = {}


def _build_nc():
    import concourse.bass as bass
    import concourse.bacc as bacc
    import concourse.mybir as mybir
    import concourse.tile as tile
    from concourse import masks

    f32 = mybir.dt.float32
    bf16 = mybir.dt.bfloat16
    nc = bacc.Bacc(
        "TRN2",
        target_bir_lowering=False,
        debug=False,
        enable_asserts=False,
        num_devices=NCORES,
    )
    attn = nc.dram_tensor("attn", [Hh, QPC, T], f32, kind="ExternalInput").ap()
    mlp = nc.dram_tensor("mlp", [T, H], f32, kind="ExternalInput").ap()
    out = nc.dram_tensor("out", [QPC, H], f32, kind="ExternalOutput").ap()

    with tile.TileContext(nc) as tc:
        with (
            tc.tile_pool(name="persist", bufs=1) as persist,
            tc.tile_pool(name="heads", bufs=6) as heads,
            tc.tile_pool(name="heads1", bufs=2) as heads1,
            tc.tile_pool(name="acc", bufs=3) as accp,
            tc.tile_pool(name="stage", bufs=2) as stagep,
            tc.tile_pool(name="wm", bufs=1) as wmp,
            tc.tile_pool(name="wmt", bufs=1) as wmtp,
            tc.tile_pool(name="small", bufs=2) as small,
            tc.tile_pool(name="outsb", bufs=2) as outsbp,
            tc.tile_pool(name="tp_psum", bufs=2, space="PSUM") as tp_psum,
            tc.tile_pool(name="mm_psum", bufs=2, space="PSUM") as mm_psum,
        ):
            mlp_sb = persist.tile([P, S_CHUNKS, H], bf16)
            ident = persist.tile([P, P], f32)
            masks.make_identity(nc, ident[:])
            ident_bf = persist.tile([P, P], bf16)
            masks.make_identity(nc, ident_bf[:])

            mlp_next = [0]

            def load_mlp_group(n):
                # fp32 chunks staged via HWDGE, cast bf16 on ScalarE
                for _ in range(n):
                    c = mlp_next[0]
                    if c >= S_CHUNKS:
                        return
                    mlp_next[0] += 1
                    st = stagep.tile([P, H], f32, tag="st")
                    nc.sync.dma_start(out=st, in_=mlp[c * P : (c + 1) * P, :])
                    nc.scalar.copy(mlp_sb[:, c, :], st)

            def transpose_half(wm, wmt, half):
                for g in range(2 * half, 2 * half + 2):
                    pt = tp_psum.tile([P, 4 * P], bf16, tag="ptb")
                    for j in range(4):
                        c = 4 * g + j
                        nc.tensor.transpose(
                            pt[:, j * P : (j + 1) * P],
                            wm[:, c * P : (c + 1) * P],
                            ident_bf[:],
                        )
                    nc.scalar.copy(wmt[:, 4 * g : 4 * g + 4, :], pt[:])

            def epilogue(acc, mx, q, tail=False):
                # mask with the row's 8th-largest (two s-halves so the
                # TensorEngine starts transposing early), renormalize,
                # transpose, contract, store.
                HF = T // 2
                v8 = mx[:, K - 1 : K]
                wm = wmp.tile([P, T], bf16, tag="wm")
                wmt = wmtp.tile([P, S_CHUNKS, P], bf16, tag="wmt")
                stot = small.tile([P, 1], f32, tag="stot")
                if tail:
                    # halved mask so transposes/matmuls start 1.2us earlier
                    ssum = small.tile([P, 2], f32, tag="ssum")
                    for half in range(2):
                        sl = slice(half * HF, (half + 1) * HF)
                        nc.vector.scalar_tensor_tensor(
                            out=wm[:, sl],
                            in0=acc[:, sl],
                            scalar=v8,
                            in1=acc[:, sl],
                            op0=mybir.AluOpType.is_ge,
                            op1=mybir.AluOpType.mult,
                            accum_out=ssum[:, half : half + 1],
                        )
                        transpose_half(wm, wmt, half)
                    nc.vector.tensor_add(
                        out=stot, in0=ssum[:, 0:1], in1=ssum[:, 1:2]
                    )
                else:
                    nc.vector.scalar_tensor_tensor(
                        out=wm,
                        in0=acc,
                        scalar=v8,
                        in1=acc,
                        op0=mybir.AluOpType.is_ge,
                        op1=mybir.AluOpType.mult,
                        accum_out=stot,
                    )
                    transpose_half(wm, wmt, 0)
                    transpose_half(wm, wmt, 1)
                nc.vector.tensor_scalar_max(stot, stot, float(EPS_SUM))
                rcp = small.tile([P, 1], f32, tag="rcp")
                nc.vector.reciprocal(rcp, stot)

                ps0 = mm_psum.tile([P, 512], f32, tag="ps0")
                ps1 = mm_psum.tile([P, 512], f32, tag="ps1")
                ps = [ps0, ps1]
                osb0 = outsbp.tile([P, 512], f32, tag="osb0")
                osb1 = outsbp.tile([P, 512], f32, tag="osb1")
                osb = [osb0, osb1]

                def evict(nh):
                    nsl = slice(nh * 512, (nh + 1) * 512)
                    nc.scalar.activation(
                        out=osb[nh][:, :],
                        in_=ps[nh][:, :],
                        func=mybir.ActivationFunctionType.Copy,
                        scale=rcp[:, :],
                    )
                    nc.scalar.dma_start(out=out[q, nsl], in_=osb[nh][:, :])

                if tail:
                    # nh-major: first output half evicts + stores while the
                    # second half's matmuls run (separate PSUM tiles per half
                    # so the second half's start isn't fenced on the first
                    # half's eviction read)
                    for nh in range(H // 512):
                        nsl = slice(nh * 512, (nh + 1) * 512)
                        for c in range(S_CHUNKS):
                            nc.tensor.matmul(
                                ps[nh][:, :],
                                lhsT=wmt[:, c, :],
                                rhs=mlp_sb[:, c, nsl],
                                start=(c == 0),
                                stop=(c == S_CHUNKS - 1),
                            )
                        evict(nh)
                else:
                    for c in range(S_CHUNKS):
                        for nh in range(H // 512):
                            nsl = slice(nh * 512, (nh + 1) * 512)
                            nc.tensor.matmul(
                                ps[nh][:, :],
                                lhsT=wmt[:, c, :],
                                rhs=mlp_sb[:, c, nsl],
                                start=(c == 0),
                                stop=(c == S_CHUNKS - 1),
                            )
                    for nh in range(H // 512):
                        evict(nh)

            pending = []  # deferred (acc, mx, q) epilogues

            def run_pending():
                if pending:
                    epilogue(*pending.pop())

            def accumulate(acc, q, sl, mlp_every=0, warm_pe=False, upto=Hh):
                # sequential h order on DVE: bit-exact with the reference
                # mean's accumulation order, so top-8 selection matches.
                # Heads arrive PAIRED in one 2 MiB DMA (halves the DVE's
                # DMA-completion semaphore waits and the Sync trigger count);
                # the two adds still run in h order from the shared buffer.
                n = sl.stop - sl.start
                nc.sync.dma_start(out=acc[:, sl], in_=attn[0, q, sl])
                for hp in range(1, upto - 1, 2):
                    ht2 = heads.tile([P, 2, n], f32, tag="ht2")
                    nc.sync.dma_start(
                        out=ht2,
                        in_=attn[hp : hp + 2, q, sl].transpose([1, 0, 2]),
                    )
                    for j in range(2):
                        nc.vector.tensor_add(
                            out=acc[:, sl], in0=acc[:, sl], in1=ht2[:, j, :]
                        )
                    if warm_pe and hp >= 7:
                        # keep the PE HAM window busy through the tail: one
                        # throwaway transpose per arriving pair (dep = its
                        # DMA, so they space out with the stream)
                        pt = tp_psum.tile([P, 4 * P], f32, tag="pt")
                        nc.tensor.transpose(pt[:, 0:P], ht2[:, 0, 0:P], ident[:])
                    if mlp_every:
                        load_mlp_group(2)
                if (upto - 1) % 2 == 1:
                    # odd remaining head (h = upto-1 = 15 for full tiles)
                    ht = heads1.tile([P, n], f32, tag="ht")
                    nc.sync.dma_start(out=ht, in_=attn[upto - 1, q, sl])
                    nc.vector.tensor_add(
                        out=acc[:, sl], in0=acc[:, sl], in1=ht
                    )

            for t in range(TQ_TILES - 1):
                q = slice(t * P, (t + 1) * P)
                acc = accp.tile([P, T], f32, tag="acc")
                # interleave the 16 mlp chunk loads into tiles 0-1 in groups
                # of 4 (= stage bufs), finishing well before tile 0's matmuls
                accumulate(acc, q, slice(0, T), mlp_every=2 if t < 2 else 0)
                mx = small.tile([P, K], f32, tag="mx")
                nc.vector.max(out=mx, in_=acc)
                run_pending()
                pending.append((acc, mx, q))

            # last tile: stream s-halves; the first half's top-8 scan and the
            # second-to-last tile's epilogue hide under the second half's
            # streaming, so only this tile's epilogue trails the stream
            t = TQ_TILES - 1
            q = slice(t * P, (t + 1) * P)
            HF = T // 2
            acc = accp.tile([P, T], f32, tag="acc")
            accumulate(acc, q, slice(0, HF))
            mg = small.tile([P, 3 * K], f32, tag="mg")
            nc.vector.max(out=mg[:, :K], in_=acc[:, :HF])
            run_pending()
            accumulate(acc, q, slice(HF, T), warm_pe=True, upto=Hh - 1)
            # final head in quarter-columns, quarter max8s interleaved, so
            # the post-stream serial chain is as short as possible
            QQ = HF // 2
            htl = heads1.tile([P, HF], f32, tag="ht")
            nc.sync.dma_start(out=htl[:, :QQ], in_=attn[Hh - 1, q, HF : HF + QQ])
            nc.sync.dma_start(out=htl[:, QQ:], in_=attn[Hh - 1, q, HF + QQ :])
            nc.vector.tensor_add(
                out=acc[:, HF : HF + QQ], in0=acc[:, HF : HF + QQ], in1=htl[:, :QQ]
            )
            nc.vector.max(out=mg[:, K : 2 * K], in_=acc[:, HF : HF + QQ])
            ptw = tp_psum.tile([P, 4 * P], f32, tag="pt")
            nc.tensor.transpose(ptw[:, 0:P], acc[:, HF : HF + P], ident[:])
            nc.vector.tensor_add(
                out=acc[:, HF + QQ :], in0=acc[:, HF + QQ :], in1=htl[:, QQ:]
            )
            nc.vector.max(out=mg[:, 2 * K :], in_=acc[:, HF + QQ :])
            g8 = small.tile([P, K], f32, tag="g8")
            nc.vector.max(out=g8, in_=mg)
            ptw2 = tp_psum.tile([P, 4 * P], f32, tag="pt")
            nc.tensor.transpose(ptw2[:, 0:P], acc[:, HF : HF + P], ident[:])
            epilogue(acc, g8, q, tail=True)

    nc.compile()
    return nc


def _get_nc():
    if "nc" not in _compiled:
        _compiled["nc"] = _build_nc()
    return _compiled["nc"]


def kernel(mlp_hidden: np.ndarray, attn_weights: np.ndarray) -> np.ndarray:
    from concourse.bass_utils import run_bass_kernel_spmd

    mlp_hidden = np.ascontiguousarray(mlp_hidden, dtype=np.float32)
    attn_weights = np.ascontiguousarray(attn_weights, dtype=np.float32)
    assert mlp_hidden.shape == (B, T, H)
    assert attn_weights.shape == (B, Hh, T, T)

    nc = _get_nc()
    in_maps = []
    for c in range(NCORES):
        b = c // (NCORES // B)
        q0 = (c % (NCORES // B)) * QPC
        in_maps.append(
            {
                "attn": np.ascontiguousarray(attn_weights[b, :, q0 : q0 + QPC, :]),
                "mlp": mlp_hidden[b],
            }
        )
    res = run_bass_kernel_spmd(nc, in_maps, list(range(NCORES)))
    out = np.empty((B, T, H), dtype=np.float32)
    for c in range(NCORES):
        b = c // (NCORES // B)
        q0 = (c % (NCORES // B)) * QPC
        out[b, q0 : q0 + QPC] = res.results[c]["out"]
    return out



# revision 3
# speedup vs baseline: 1.0242x; 1.0242x over previous
"""Trainium2 Bass kernel for CrossTokenMLPAggregator (top-k masked attention aggregation).

Computes, for full inputs
    mlp_hidden   [B=2, T=2048, H=1024] f32
    attn_weights [B=2, Hh=16, T=2048, T=2048] f32
the reference:
    W = attn_weights.mean(axis=1)              # [B, T, T]
    keep top-8 per query row, renormalize kept mass to sum 1
    out = einsum('bts,bsh->bth', W_sparse, mlp_hidden)

Sharding: 8 cores, each owns 512 query rows (core c -> batch c//4,
query rows (c%4)*512 ...). Each core streams its [16, 512, 2048] slice of
attn_weights (the 512 MiB input dominates; the split is exact), sums the
heads sequentially on DVE (bit-exact with the reference's mean
accumulation order, so the top-8 selection matches exactly), finds
the top-8 with the DVE max8 instruction, masks with (W >= v8)*W in one
scalar_tensor_tensor op, transposes the masked rows on the TensorEngine
and contracts in bf16 (fp32 PSUM accumulate; ~0.2% rounding, far inside
the 2e-2 budget) against the bf16 mlp_hidden slice resident in SBUF.
Renormalization (1/kept fp32 mass) rides the PSUM->SBUF eviction on the
ScalarEngine; outputs are stored bf16 and upcast to f32 on the host
(~0.1% extra rounding).

Schedule notes (profiled: DMA queue busy ~99%, stream sustains
~420 GB/s with 2 MiB DMAs; 1 MiB DMAs only reach ~340 GB/s; DVE
tensor_tensor f32 runs ~114 G elem/s, ~2.3 us per full-width add, with
~1.8 us of slack per head pair, so the attn stream paces everything):
- Per-tile epilogues (mask/transpose/matmul/store) are software-pipelined
  one tile late so the add chain - whose head-buffer releases pace the
  attn stream - never pauses at tile boundaries.
- Heads arrive PAIRED in 2 MiB DMAs for ALL four tiles (the previous
  halved last tile used 1 MiB DMAs and dropped to ~340 GB/s for its
  whole 16.8 MiB: ~4-5 us lost). Only the final head (h15) of the last
  tile streams in 256 KiB quarter-columns with the quarter top-8 scans
  interleaved, so the post-stream serial chain is one quarter add + one
  quarter max8 + combine before masking can start.
- The mlp load is staged fp32 on the SCALAR HWDGE ring (16 x 512 KiB,
  3 stage bufs) and cast to bf16 on the ScalarEngine; this keeps the
  Sync ring pure attn (no head-of-line sharing, fewer Sync sequencer
  issue slots) and the two HWDGE rings round-robin at packet granularity
  so total bytes are unchanged. Casts finish ~38 us, before tile 0's
  matmuls need the chunks.
- Output DMAs issue from the ScalarEngine's HWDGE queue right after the
  eviction they depend on.
- Idle TensorEngine cycles during the last tile's stream get keep-warm
  transposes so the HAM clock gate doesn't re-throttle the PE right
  before the tail matmul block (HAM k=4 windows halve PE rate but leave
  the DMA stream untouched); the tail matmuls run nh-major so the first
  output half stores while the second computes.
"""

import numpy as np

B, T, H, Hh, K = 2, 2048, 1024, 16, 8
NCORES = 8
QPC = (B * T) // NCORES          # 512 query rows per core
P = 128                          # partitions
TQ_TILES = QPC // P              # 4 tiles of 128 query rows
S_CHUNKS = T // P                # 16 contraction chunks
EPS_SUM = np.float32(1e-8) * np.float32(16.0)  # EPS in head-sum domain

_compiled = {}


def _build_nc():
    import concourse.bass as bass
    import concourse.bacc as bacc
    import concourse.mybir as mybir
    import concourse.tile as tile
    from concourse import masks

    f32 = mybir.dt.float32
    bf16 = mybir.dt.bfloat16
    nc = bacc.Bacc(
        "TRN2",
        target_bir_lowering=False,
        debug=False,
        enable_asserts=False,
        num_devices=NCORES,
    )
    attn = nc.dram_tensor("attn", [Hh, QPC, T], f32, kind="ExternalInput").ap()
    mlp = nc.dram_tensor("mlp", [T, H], f32, kind="ExternalInput").ap()
    out = nc.dram_tensor("out", [QPC, H], bf16, kind="ExternalOutput").ap()

    with tile.TileContext(nc) as tc:
        with (
            tc.tile_pool(name="persist", bufs=1) as persist,
            tc.tile_pool(name="heads", bufs=6) as heads,
            tc.tile_pool(name="heads1", bufs=2) as heads1,
            tc.tile_pool(name="acc", bufs=3) as accp,
            tc.tile_pool(name="stage", bufs=3) as stagep,
            tc.tile_pool(name="wm", bufs=1) as wmp,
            tc.tile_pool(name="wmt", bufs=1) as wmtp,
            tc.tile_pool(name="small", bufs=2) as small,
            tc.tile_pool(name="outsb", bufs=2) as outsbp,
            tc.tile_pool(name="tp_psum", bufs=2, space="PSUM") as tp_psum,
            tc.tile_pool(name="mm_psum", bufs=2, space="PSUM") as mm_psum,
        ):
            mlp_sb = persist.tile([P, S_CHUNKS, H], bf16)
            ident = persist.tile([P, P], f32)
            masks.make_identity(nc, ident[:])
            ident_bf = persist.tile([P, P], bf16)
            masks.make_identity(nc, ident_bf[:])

            def load_mlp_all():
                # fp32 chunks staged via the Scalar HWDGE ring, cast bf16
                # on ScalarE (3 stage bufs pipeline DMA vs cast)
                for c in range(S_CHUNKS):
                    st = stagep.tile([P, H], f32, tag="st")
                    nc.scalar.dma_start(out=st, in_=mlp[c * P : (c + 1) * P, :])
                    nc.scalar.copy(mlp_sb[:, c, :], st)

            def transpose_half(wm, wmt, half):
                for g in range(2 * half, 2 * half + 2):
                    pt = tp_psum.tile([P, 4 * P], bf16, tag="ptb")
                    for j in range(4):
                        c = 4 * g + j
                        nc.tensor.transpose(
                            pt[:, j * P : (j + 1) * P],
                            wm[:, c * P : (c + 1) * P],
                            ident_bf[:],
                        )
                    nc.scalar.copy(wmt[:, 4 * g : 4 * g + 4, :], pt[:])

            def epilogue(acc, mx, q, tail=False):
                # mask with the row's 8th-largest (two s-halves so the
                # TensorEngine starts transposing early), renormalize,
                # transpose, contract, store.
                HF = T // 2
                v8 = mx[:, K - 1 : K]
                wm = wmp.tile([P, T], bf16, tag="wm")
                wmt = wmtp.tile([P, S_CHUNKS, P], bf16, tag="wmt")
                stot = small.tile([P, 1], f32, tag="stot")
                if tail:
                    # halved mask so transposes/matmuls start ~1.2us earlier
                    ssum = small.tile([P, 2], f32, tag="ssum")
                    for half in range(2):
                        sl = slice(half * HF, (half + 1) * HF)
                        nc.vector.scalar_tensor_tensor(
                            out=wm[:, sl],
                            in0=acc[:, sl],
                            scalar=v8,
                            in1=acc[:, sl],
                            op0=mybir.AluOpType.is_ge,
                            op1=mybir.AluOpType.mult,
                            accum_out=ssum[:, half : half + 1],
                        )
                        transpose_half(wm, wmt, half)
                    nc.vector.tensor_add(
                        out=stot, in0=ssum[:, 0:1], in1=ssum[:, 1:2]
                    )
                else:
                    nc.vector.scalar_tensor_tensor(
                        out=wm,
                        in0=acc,
                        scalar=v8,
                        in1=acc,
                        op0=mybir.AluOpType.is_ge,
                        op1=mybir.AluOpType.mult,
                        accum_out=stot,
                    )
                    transpose_half(wm, wmt, 0)
                    transpose_half(wm, wmt, 1)
                nc.vector.tensor_scalar_max(stot, stot, float(EPS_SUM))
                rcp = small.tile([P, 1], f32, tag="rcp")
                nc.vector.reciprocal(rcp, stot)

                ps0 = mm_psum.tile([P, 512], f32, tag="ps0")
                ps1 = mm_psum.tile([P, 512], f32, tag="ps1")
                ps = [ps0, ps1]
                osb0 = outsbp.tile([P, 512], bf16, tag="osb0")
                osb1 = outsbp.tile([P, 512], bf16, tag="osb1")
                osb = [osb0, osb1]

                def evict(nh):
                    nsl = slice(nh * 512, (nh + 1) * 512)
                    nc.scalar.activation(
                        out=osb[nh][:, :],
                        in_=ps[nh][:, :],
                        func=mybir.ActivationFunctionType.Copy,
                        scale=rcp[:, :],
                    )
                    nc.scalar.dma_start(out=out[q, nsl], in_=osb[nh][:, :])

                if tail:
                    # nh-major: first output half evicts + stores while the
                    # second half's matmuls run (separate PSUM tiles per half
                    # so the second half's start isn't fenced on the first
                    # half's eviction read)
                    for nh in range(H // 512):
                        nsl = slice(nh * 512, (nh + 1) * 512)
                        for c in range(S_CHUNKS):
                            nc.tensor.matmul(
                                ps[nh][:, :],
                                lhsT=wmt[:, c, :],
                                rhs=mlp_sb[:, c, nsl],
                                start=(c == 0),
                                stop=(c == S_CHUNKS - 1),
                            )
                        evict(nh)
                else:
                    for c in range(S_CHUNKS):
                        for nh in range(H // 512):
                            nsl = slice(nh * 512, (nh + 1) * 512)
                            nc.tensor.matmul(
                                ps[nh][:, :],
                                lhsT=wmt[:, c, :],
                                rhs=mlp_sb[:, c, nsl],
                                start=(c == 0),
                                stop=(c == S_CHUNKS - 1),
                            )
                    for nh in range(H // 512):
                        evict(nh)

            pending = []  # deferred (acc, mx, q) epilogues

            def run_pending():
                if pending:
                    epilogue(*pending.pop())

            def accumulate(acc, q, sl, warm_pe=False, upto=Hh, mid_cb=None):
                # sequential h order on DVE: bit-exact with the reference
                # mean's accumulation order, so top-8 selection matches.
                # Heads arrive PAIRED in one 2 MiB DMA (sustains ~420 GB/s
                # where 1 MiB only reaches ~340); the very first pair is
                # fused into acc with one two-operand add (acc = h0 + h1,
                # identical arithmetic order), so EVERY stream DMA is a
                # full 2 MiB pair.
                n = sl.stop - sl.start
                first = True
                for hp in range(0, upto - 1, 2):
                    ht2 = heads.tile([P, 2, n], f32, tag="ht2")
                    nc.sync.dma_start(
                        out=ht2,
                        in_=attn[hp : hp + 2, q, sl].transpose([1, 0, 2]),
                    )
                    if first:
                        nc.vector.tensor_add(
                            out=acc[:, sl], in0=ht2[:, 0, :], in1=ht2[:, 1, :]
                        )
                        first = False
                    else:
                        for j in range(2):
                            nc.vector.tensor_add(
                                out=acc[:, sl], in0=acc[:, sl], in1=ht2[:, j, :]
                            )
                    if warm_pe and hp >= 6:
                        # keep the PE HAM window busy through the tail: one
                        # throwaway transpose per arriving pair (dep = its
                        # DMA, so they space out with the stream)
                        pt = tp_psum.tile([P, 4 * P], f32, tag="pt")
                        nc.tensor.transpose(pt[:, 0:P], ht2[:, 0, 0:P], ident[:])
                    if mid_cb is not None and hp == 6:
                        # pipelined previous-tile epilogue, emitted mid-chain
                        # so its DVE ops don't delay the tail quarter scans
                        mid_cb()
                if upto % 2 == 1:
                    # odd trailing head (h14 when upto=15 for the last tile)
                    ht = heads1.tile([P, n], f32, tag="ht")
                    nc.sync.dma_start(out=ht, in_=attn[upto - 1, q, sl])
                    nc.vector.tensor_add(
                        out=acc[:, sl], in0=acc[:, sl], in1=ht
                    )

            load_mlp_all()

            for t in range(TQ_TILES - 1):
                q = slice(t * P, (t + 1) * P)
                acc = accp.tile([P, T], f32, tag="acc")
                accumulate(acc, q, slice(0, T))
                mx = small.tile([P, K], f32, tag="mx")
                nc.vector.max(out=mx, in_=acc)
                run_pending()
                pending.append((acc, mx, q))

            # last tile: full-row paired streaming like the others (2 MiB
            # DMAs sustain ~420 GB/s; halved 1 MiB DMAs only ~340), except
            # the final head h15 streams in quarter-columns with quarter
            # max8 scans interleaved so the post-stream serial chain is
            # just one quarter add + quarter max8 + combine.
            t = TQ_TILES - 1
            q = slice(t * P, (t + 1) * P)
            acc = accp.tile([P, T], f32, tag="acc")
            accumulate(acc, q, slice(0, T), warm_pe=True, upto=Hh - 1,
                       mid_cb=run_pending)
            QQ = T // 4
            mg = small.tile([P, 4 * K], f32, tag="mg")
            htl = heads1.tile([P, T], f32, tag="ht")
            for j in range(4):
                qsl = slice(j * QQ, (j + 1) * QQ)
                nc.sync.dma_start(out=htl[:, qsl], in_=attn[Hh - 1, q, qsl])
                nc.vector.tensor_add(
                    out=acc[:, qsl], in0=acc[:, qsl], in1=htl[:, qsl]
                )
                nc.vector.max(out=mg[:, j * K : (j + 1) * K], in_=acc[:, qsl])
                if j == 1 or j == 3:
                    # keep-warm transposes spaced through the quarter chain
                    ptw = tp_psum.tile([P, 4 * P], f32, tag="pt")
                    nc.tensor.transpose(ptw[:, 0:P], acc[:, 0:P], ident[:])
            g8 = small.tile([P, K], f32, tag="g8")
            nc.vector.max(out=g8, in_=mg)
            epilogue(acc, g8, q, tail=True)

    nc.compile()
    return nc


def _get_nc():
    if "nc" not in _compiled:
        _compiled["nc"] = _build_nc()
    return _compiled["nc"]


def kernel(mlp_hidden: np.ndarray, attn_weights: np.ndarray) -> np.ndarray:
    from concourse.bass_utils import run_bass_kernel_spmd

    mlp_hidden = np.ascontiguousarray(mlp_hidden, dtype=np.float32)
    attn_weights = np.ascontiguousarray(attn_weights, dtype=np.float32)
    assert mlp_hidden.shape == (B, T, H)
    assert attn_weights.shape == (B, Hh, T, T)

    nc = _get_nc()
    in_maps = []
    for c in range(NCORES):
        b = c // (NCORES // B)
        q0 = (c % (NCORES // B)) * QPC
        in_maps.append(
            {
                "attn": np.ascontiguousarray(attn_weights[b, :, q0 : q0 + QPC, :]),
                "mlp": mlp_hidden[b],
            }
        )
    res = run_bass_kernel_spmd(nc, in_maps, list(range(NCORES)))
    out = np.empty((B, T, H), dtype=np.float32)
    for c in range(NCORES):
        b = c // (NCORES // B)
        q0 = (c % (NCORES // B)) * QPC
        out[b, q0 : q0 + QPC] = res.results[c]["out"].astype(np.float32)
    return out
